# revision 22
# baseline (speedup 1.0000x reference)
"""Gaussian square-sensor splat on 8 Trainium2 NeuronCores (v5).

Decomposition: the 2048x2048 image is split into 64x64=4096 blocks of
32x32 px.  Each block is assigned to one of 8 cores by COUNT-BALANCED
DEALING: blocks sorted by point count, rank r -> core r%8, slot r//8.
Slot capacities are 64-quantized; 8 slots pack into each PSUM strip.

v5 layout: per chunk the work tensors are [P, 2*gc, 18, 2] with the
taps (36 = 18*2) CONTIGUOUS per (dim, column) slab:
  - matmul operands pr/colp slabs [k, 36] are contiguous (fast PE)
  - every DVE operand still ends in a packed [1,2] dim, so the DVE
    2x_1p perf mode engages: broadcasts (dc over taps, vn over taps)
    are expressed via PAIR-DUPLICATED host inputs dcq2/vnb2 whose AP
    last dim is [stride 1, count 2]
  - d = iota16 - dcq2          int16 fixed point (1/256 px), DVE @2x
  - pr = D_ERF(sqrt2/256 * d)  ACT engine, bf16, packed read/write
  - colp = pr[col] * vnb2      bf16 @2x, split DVE/Pool for balance
  - strip matmuls on PE accumulate 8 slots per [36, 288] PSUM strip;
    accumulation groups are reordered to START at partition base 0
    (HW crash otherwise); rare all-offset slots open with a k=1 zero
    matmul
  - strip PSUM->SBUF copies on DVE, DMA out from SBUF
int16 keeps d exact (bf16/fp16 coords would lose 0.02-0.14 px).
Host precomputes quantized patch offsets and theta-normalized values
vn = v / (2 (1+2q cos 2pi fy)(1+2q cos 2pi fx)).
"""
import math
import sys

sys.path.insert(0, '/opt/trn_rl_repo')

import numpy as np

WIDTH = HEIGHT = 2048
N_POINTS = 1 << 20
N_CORES = 8
BLK = 32
PW = 36
GRID = WIDTH // BLK                     # 64 blocks per side
NBLK = GRID * GRID                      # 4096
NSLOT = NBLK // N_CORES                 # 512 slots per core
NSTRIP = NSLOT // 8                     # 64 psum strips per core
P = 128
GCMAX = 64                              # max columns per chunk
CAPQ = 64                               # slot capacity quantum
                                        # (32 triggers PE 32-row-tile
                                        # transition crashes)
MUL_DVE = 0.0                           # fraction of multiply on DVE

_Q2 = 2.0 * math.exp(-math.pi ** 2 / 2.0)

_COMPILED = None          # (nc, plan, lay)


def _block_plan(x, y):
    """Assign blocks to (core, slot) by count-balanced dealing."""
    xp = (x.astype(np.float64) + 1.0) * (WIDTH / 2.0)
    yp = (y.astype(np.float64) + 1.0) * (HEIGHT / 2.0)
    xb = np.clip(np.floor(xp).astype(np.int64), 0, WIDTH - 1)
    yb = np.clip(np.floor(yp).astype(np.int64), 0, HEIGHT - 1)
    gb = (yb // BLK) * GRID + xb // BLK            # global block id
    counts = np.bincount(gb, minlength=NBLK)
    order = np.argsort(-counts, kind="stable")     # blocks by count desc
    core_of = np.empty(NBLK, np.int64)
    slot_of = np.empty(NBLK, np.int64)
    rank = np.arange(NBLK)
    core_of[order] = rank % N_CORES
    slot_of[order] = rank // N_CORES
    # slot capacity = max count within the slot's 8 blocks, CAPQ-quantized
    slot_max = counts[order].reshape(NSLOT, N_CORES).max(axis=1)
    caps = (np.ceil(slot_max / CAPQ).astype(np.int64) * CAPQ).clip(CAPQ, None)
    # inverse table: (core, slot) -> block id
    inv = np.empty((N_CORES, NSLOT), np.int64)
    inv[core_of[order], slot_of[order]] = order
    return dict(counts=counts, core_of=core_of, slot_of=slot_of,
                caps=caps, inv=inv)


def _layout_from_caps(caps):
    """Slot slab layout: slots packed per strip (8 slots/strip), strips
    padded to whole 128-slot columns.  Matmul segments: p0==64 -> k<=64.
    Each slot's segment list is rotated so a p0==0 segment (if any)
    comes first and carries start=True; slots with no p0==0 segment are
    marked need_zero (opened by a k=1 zero matmul)."""
    strip_cols = np.zeros(NSTRIP, np.int64)
    slot_off = np.zeros(NSLOT, np.int64)     # slot offset within strip
    jpos = np.zeros(NSLOT, np.int64)         # position of slot in strip
    col_base = np.zeros(NSTRIP, np.int64)
    segs = []
    for s in range(NSTRIP):
        off = 0
        for j in range(8):
            sl = s * 8 + j
            slot_off[sl] = off
            jpos[sl] = j
            off += int(caps[sl])
        strip_cols[s] = (off + 127) // 128
    col_base[1:] = np.cumsum(strip_cols)[:-1]
    F = int(strip_cols.sum())

    nzero = 0
    for s in range(NSTRIP):
        slist = []
        for sl in range(s * 8, s * 8 + 8):
            pos = int(slot_off[sl])
            rem = int(caps[sl])
            j = int(jpos[sl])
            parts = []
            while rem > 0:
                t = pos // 128
                p0 = pos % 128
                assert p0 in (0, 64), f"illegal partition base {p0}"
                k = min(128 - p0, rem)
                if p0 == 64:
                    k = min(k, 64)
                pos += k
                rem -= k
                parts.append((t, p0, k))
            # rotate a p0==0 part to the front (group must start at
            # partition base 0)
            i0 = next((i for i, e in enumerate(parts) if e[1] == 0), None)
            if i0 is None:
                nzero += 1
                need_zero = True
            else:
                parts = parts[i0:i0 + 1] + parts[:i0] + parts[i0 + 1:]
                need_zero = False
            n = len(parts)
            for i, (t, p0, k) in enumerate(parts):
                first = (i == 0) and not need_zero
                slist.append((t, p0, k, j, first, i == n - 1,
                              need_zero and i == 0))
        segs.append(slist)

    chunks = []
    s0 = 0
    while s0 < NSTRIP:
        s1 = s0
        cols = 0
        while s1 < NSTRIP and cols + strip_cols[s1] <= GCMAX:
            cols += strip_cols[s1]
            s1 += 1
        assert s1 > s0, "single strip exceeds GCMAX"
        chunks.append((s0, s1, int(col_base[s0]), int(cols)))
        s0 = s1
    return dict(slot_off=slot_off, strip_cols=strip_cols, col_base=col_base,
                jpos=jpos, F=F, segs=segs, chunks=chunks, nzero=nzero)


def _build_program(lay):
    import concourse.bacc as bacc
    import concourse.mybir as mybir
    from concourse.tile import TileContext

    dt = mybir.dt
    Act = mybir.ActivationFunctionType
    Alu = mybir.AluOpType

    F = lay["F"]
    nc = bacc.Bacc("TRN2", target_bir_lowering=False, debug=False)

    # dcq2: per chunk, [dcy cols | dcx cols] pair-duplicated -> [P, 2F, 2]
    dcq2 = nc.dram_tensor("dcq2", [P, 2 * F, 2], dt.int16,
                          kind="ExternalInput")
    # vnb2: F columns pair-duplicated -> [P, F, 2]
    vnb2 = nc.dram_tensor("vnb2", [P, F, 2], dt.bfloat16,
                          kind="ExternalInput")
    iot = nc.dram_tensor("iot", [P, 18, 2], dt.int16, kind="ExternalInput")
    out = nc.dram_tensor("out", [NSTRIP, PW, 8 * PW], dt.float32,
                         kind="ExternalOutput")

    SC = float(math.sqrt(2.0) / 256.0)

    with TileContext(nc) as tc:
        with (
            tc.tile_pool(name="io", bufs=1) as io,
            tc.tile_pool(name="prof", bufs=1) as prof,
            tc.tile_pool(name="stage", bufs=4) as stage,
            tc.tile_pool(name="psum", bufs=2, space="PSUM") as psum,
        ):
            t_dcq2 = io.tile([P, 2 * F, 2], dt.int16)
            t_vnb2 = io.tile([P, F, 2], dt.bfloat16)
            t_iot = io.tile([P, 18, 2], dt.int16)
            t_zro = io.tile([P, PW], dt.bfloat16)
            nc.sync.dma_start(out=t_dcq2[:], in_=dcq2[:])
            nc.sync.dma_start(out=t_vnb2[:], in_=vnb2[:])
            nc.sync.dma_start(out=t_iot[:], in_=iot[:])
            nc.gpsimd.memset(t_zro[:], 0.0)

            BANKF = 512                 # fp32 elems per PSUM bank

            def emit_mm(ck):
                """Matmuls for a chunk's strips into one 4-bank PSUM tile."""
                (s0, s1, c0, gc), pr, colp = ck
                nst = s1 - s0
                pst = psum.tile([PW, 4, BANKF], dt.float32,
                                tag="pst", name="pst")
                for si, s in enumerate(range(s0, s1)):
                    base = int(lay["col_base"][s]) - c0
                    for (t, p0, k, j, first, last, zopen) in lay["segs"][s]:
                        tl = base + t
                        if zopen:
                            nc.tensor.matmul(
                                out=pst[:, si, j * PW:(j + 1) * PW],
                                lhsT=t_zro[0:1, :],
                                rhs=t_zro[0:1, :],
                                start=True, stop=False)
                        nc.tensor.matmul(
                            out=pst[:, si, j * PW:(j + 1) * PW],
                            lhsT=pr[p0:p0 + k, tl],
                            rhs=colp[p0:p0 + k, tl],
                            start=first, stop=last)
                return (s0, nst, pst)

            ncopy = [0]

            def emit_copy(ck):
                """One batched strip copy (alternating DVE/ACT) + DMAs."""
                s0, nst, pst = ck
                st = stage.tile([PW, 4, 8 * PW], dt.float32,
                                tag="st", name="st")
                eng = nc.vector.tensor_copy if ncopy[0] % 2 == 0 \
                    else nc.scalar.copy
                eng(out=st[:, :nst], in_=pst[:, :nst, :8 * PW])
                ncopy[0] += 1
                for si in range(nst):
                    nc.sync.dma_start(out=out[s0 + si], in_=st[:, si])

            co2 = 0                     # running column offset into dcq2
            prev = None                 # software pipeline: mm lags 1 chunk
            pcopy = None                # copies lag 2 chunks
            for (s0, s1, c0, gc) in lay["chunks"]:
                d = prof.tile([P, 2 * gc, 18, 2], dt.int16, tag="d", bufs=2,
                              name=f"d{gc}")
                nc.vector.tensor_tensor(
                    out=d[:],
                    in0=t_iot[:, None, :, :].to_broadcast([P, 2 * gc, 18, 2]),
                    in1=t_dcq2[:, co2:co2 + 2 * gc, None, :]
                        .to_broadcast([P, 2 * gc, 18, 2]),
                    op=Alu.subtract)
                pr = prof.tile([P, 2 * gc, 18, 2], dt.bfloat16, tag="pr",
                               bufs=3, name=f"pr{gc}")
                nc.scalar.activation(out=pr[:], in_=d[:],
                                     func=Act.Derivative_Erf, scale=SC)
                colp = prof.tile([P, gc, 18, 2], dt.bfloat16, tag="colp",
                                 bufs=3, name=f"colp{gc}")
                # multiply split DVE/Pool for engine balance (by column)
                g1 = max(0, min(gc, int(round(gc * MUL_DVE))))
                if g1 > 0:
                    nc.vector.tensor_tensor(
                        out=colp[:, :g1], in0=pr[:, gc:gc + g1],
                        in1=t_vnb2[:, c0:c0 + g1, None, :]
                            .to_broadcast([P, g1, 18, 2]),
                        op=Alu.mult)
                if g1 < gc:
                    nc.gpsimd.tensor_tensor(
                        out=colp[:, g1:], in0=pr[:, gc + g1:2 * gc],
                        in1=t_vnb2[:, c0 + g1:c0 + gc, None, :]
                            .to_broadcast([P, gc - g1, 18, 2]),
                        op=Alu.mult)

                if pcopy is not None:
                    emit_copy(pcopy)
                    pcopy = None
                if prev is not None:
                    pcopy = emit_mm(prev)
                prev = ((s0, s1, c0, gc), pr, colp)
                co2 += 2 * gc
            if pcopy is not None:
                emit_copy(pcopy)
            emit_copy(emit_mm(prev))
    nc.compile()
    from concourse.bass_interp import get_hw_module
    nc.m = get_hw_module(nc.m)
    return nc


def _host_shard(x, y, values, plan, lay):
    from ml_dtypes import bfloat16

    xp = (x.astype(np.float64) + 1.0) * (WIDTH / 2.0)
    yp = (y.astype(np.float64) + 1.0) * (HEIGHT / 2.0)
    xb = np.clip(np.floor(xp).astype(np.int64), 0, WIDTH - 1)
    yb = np.clip(np.floor(yp).astype(np.int64), 0, HEIGHT - 1)
    bcx = xb // BLK
    bry = yb // BLK
    gb = bry * GRID + bcx
    core = plan["core_of"][gb]
    slot = plan["slot_of"][gb]
    dcxq_all = np.round((xp - (bcx * BLK - 2)) * 256).astype(np.int16)
    dcyq_all = np.round((yp - (bry * BLK - 2)) * 256).astype(np.int16)
    fxq = (dcxq_all.astype(np.int64) % 256) / 256.0
    fyq = (dcyq_all.astype(np.int64) % 256) / 256.0
    vnorm = (values.astype(np.float64)
             / (2.0 * (1.0 + _Q2 * np.cos(2 * np.pi * fxq))
                * (1.0 + _Q2 * np.cos(2 * np.pi * fyq))))

    F = lay["F"]
    # global slot slab base: strip col_base*128 + slot_off
    slab = lay["col_base"][slot // 8] * 128 + lay["slot_off"][slot]

    iota_a = np.broadcast_to((np.arange(PW, dtype=np.int16) * 256)[None, :],
                             (P, PW)).reshape(P, 18, 2).copy()

    in_maps = []
    for c in range(N_CORES):
        m = core == c
        ps = slot[m]
        order = np.argsort(ps, kind="stable")
        ps = ps[order]
        counts = np.bincount(ps, minlength=NSLOT)
        if (counts > plan["caps"]).any():
            raise RuntimeError("slot overflow vs caps")
        starts = np.zeros(NSLOT, np.int64)
        np.cumsum(counts[:-1], out=starts[1:])
        idx = np.arange(ps.size) - starts[ps]
        dst = slab[m][order] + idx

        ya = np.full(F * P, 18 * 256, np.int16)
        xa = np.full(F * P, 18 * 256, np.int16)
        va = np.zeros(F * P, np.float64)
        ya[dst] = dcyq_all[m][order]
        xa[dst] = dcxq_all[m][order]
        va[dst] = vnorm[m][order]

        yaT = ya.reshape(F, P).T            # [P, F]
        xaT = xa.reshape(F, P).T
        vaT = va.reshape(F, P).T

        # dcq2: per chunk [dcy cols | dcx cols], each value duplicated
        dcq2_a = np.empty((P, 2 * F, 2), np.int16)
        off = 0
        for (_, _, c0, gc) in lay["chunks"]:
            dcq2_a[:, off:off + gc, 0] = yaT[:, c0:c0 + gc]
            dcq2_a[:, off + gc:off + 2 * gc, 0] = xaT[:, c0:c0 + gc]
            off += 2 * gc
        dcq2_a[:, :, 1] = dcq2_a[:, :, 0]

        vnb2_a = np.empty((P, F, 2), bfloat16)
        vnb2_a[:, :, 0] = vaT.astype(bfloat16)
        vnb2_a[:, :, 1] = vnb2_a[:, :, 0]
        in_maps.append({"dcq2": dcq2_a, "vnb2": np.ascontiguousarray(vnb2_a),
                        "iot": iota_a})
    return in_maps


def _assemble(results, plan, lay):
    img = np.zeros((HEIGHT + 4, WIDTH + 4), np.float64)
    jpos = lay["jpos"]
    for c in range(N_CORES):
        strips = results[c]["out"]          # [NSTRIP, PW, 8*PW]
        for sl in range(NSLOT):
            gb = plan["inv"][c, sl]
            bry, bcx = divmod(int(gb), GRID)
            j = int(jpos[sl])
            patch = strips[sl // 8, :, j * PW:(j + 1) * PW]
            img[bry * BLK:bry * BLK + PW, bcx * BLK:bcx * BLK + PW] += patch
    return img[2:2 + HEIGHT, 2:2 + WIDTH].astype(np.float32)


def kernel(x, y, values):
    global _COMPILED
    if _COMPILED is None:
        plan = _block_plan(x, y)
        lay = _layout_from_caps(plan["caps"])
        nc = _build_program(lay)
        _COMPILED = (nc, plan, lay)
    nc, plan, lay = _COMPILED
    in_maps = _host_shard(x, y, values, plan, lay)
    from concourse.bass_utils import run_bass_kernel_spmd
    import os
    trace = bool(int(os.environ.get("SPLAT_TRACE", "0")))
    res = run_bass_kernel_spmd(nc, in_maps, list(range(N_CORES)), trace=trace)
    kernel.last_exec_time_ns = res.exec_time_ns
    kernel.last_results = res
    return _assemble(res.results, plan, lay)


kernel.last_exec_time_ns = None


# revision 28
# speedup vs baseline: 1.0600x; 1.0600x over previous
"""Gaussian square-sensor splat on 8 Trainium2 NeuronCores (v5).

Decomposition: the 2048x2048 image is split into 64x64=4096 blocks of
32x32 px.  Each block is assigned to one of 8 cores by COUNT-BALANCED
DEALING: blocks sorted by point count, rank r -> core r%8, slot r//8.
Slot capacities are 64-quantized; 8 slots pack into each PSUM strip.

v5 layout: per chunk the work tensors are [P, 2*gc, 18, 2] with the
taps (36 = 18*2) CONTIGUOUS per (dim, column) slab:
  - matmul operands pr/colp slabs [k, 36] are contiguous (fast PE)
  - every DVE operand still ends in a packed [1,2] dim, so the DVE
    2x_1p perf mode engages: broadcasts (dc over taps, vn over taps)
    are expressed via PAIR-DUPLICATED host inputs dcq2/vnb2 whose AP
    last dim is [stride 1, count 2]
  - d = iota16 - dcq2          int16 fixed point (1/256 px), DVE @2x
  - pr = D_ERF(sqrt2/256 * d)  ACT engine, bf16, packed read/write
  - colp = pr[col] * vnb2      bf16 @2x, split DVE/Pool for balance
  - strip matmuls on PE accumulate 8 slots per [36, 288] PSUM strip;
    accumulation groups are reordered to START at partition base 0
    (HW crash otherwise); rare all-offset slots open with a k=1 zero
    matmul
  - strip PSUM->SBUF copies on DVE, DMA out from SBUF
int16 keeps d exact (bf16/fp16 coords would lose 0.02-0.14 px).
Host precomputes quantized patch offsets and theta-normalized values
vn = v / (2 (1+2q cos 2pi fy)(1+2q cos 2pi fx)).
"""
import math
import sys

sys.path.insert(0, '/opt/trn_rl_repo')

import numpy as np

WIDTH = HEIGHT = 2048
N_POINTS = 1 << 20
N_CORES = 8
BLK = 32
PW = 36
GRID = WIDTH // BLK                     # 64 blocks per side
NBLK = GRID * GRID                      # 4096
NSLOT = NBLK // N_CORES                 # 512 slots per core
NSTRIP = NSLOT // 8                     # 64 psum strips per core
P = 128
GCMAX = 64                              # max columns per chunk
CAPQ = 128                              # slot capacity quantum: all
                                        # matmuls identical 128-row
                                        # (smaller quanta trigger PE
                                        # tile-transition crashes and
                                        # 2.3x slower 64-row matmuls)
MUL_DVE = 0.15                          # fraction of multiply on DVE

_Q2 = 2.0 * math.exp(-math.pi ** 2 / 2.0)

_COMPILED = None          # (nc, plan, lay)


def _block_plan(x, y):
    """Assign blocks to (core, slot) by count-balanced dealing."""
    xp = (x.astype(np.float64) + 1.0) * (WIDTH / 2.0)
    yp = (y.astype(np.float64) + 1.0) * (HEIGHT / 2.0)
    xb = np.clip(np.floor(xp).astype(np.int64), 0, WIDTH - 1)
    yb = np.clip(np.floor(yp).astype(np.int64), 0, HEIGHT - 1)
    gb = (yb // BLK) * GRID + xb // BLK            # global block id
    counts = np.bincount(gb, minlength=NBLK)
    order = np.argsort(-counts, kind="stable")     # blocks by count desc
    core_of = np.empty(NBLK, np.int64)
    slot_of = np.empty(NBLK, np.int64)
    rank = np.arange(NBLK)
    core_of[order] = rank % N_CORES
    slot_of[order] = rank // N_CORES
    # slot capacity = max count within the slot's 8 blocks, CAPQ-quantized
    slot_max = counts[order].reshape(NSLOT, N_CORES).max(axis=1)
    caps = (np.ceil(slot_max / CAPQ).astype(np.int64) * CAPQ).clip(CAPQ, None)
    # inverse table: (core, slot) -> block id
    inv = np.empty((N_CORES, NSLOT), np.int64)
    inv[core_of[order], slot_of[order]] = order
    return dict(counts=counts, core_of=core_of, slot_of=slot_of,
                caps=caps, inv=inv)


def _layout_from_caps(caps):
    """Slot slab layout: slots packed per strip (8 slots/strip), strips
    padded to whole 128-slot columns.  Matmul segments: p0==64 -> k<=64.
    Each slot's segment list is rotated so a p0==0 segment (if any)
    comes first and carries start=True; slots with no p0==0 segment are
    marked need_zero (opened by a k=1 zero matmul)."""
    strip_cols = np.zeros(NSTRIP, np.int64)
    slot_off = np.zeros(NSLOT, np.int64)     # slot offset within strip
    jpos = np.zeros(NSLOT, np.int64)         # position of slot in strip
    col_base = np.zeros(NSTRIP, np.int64)
    segs = []
    for s in range(NSTRIP):
        off = 0
        for j in range(8):
            sl = s * 8 + j
            slot_off[sl] = off
            jpos[sl] = j
            off += int(caps[sl])
        strip_cols[s] = (off + 127) // 128
    col_base[1:] = np.cumsum(strip_cols)[:-1]
    F = int(strip_cols.sum())

    nzero = 0
    for s in range(NSTRIP):
        slist = []
        for sl in range(s * 8, s * 8 + 8):
            pos = int(slot_off[sl])
            rem = int(caps[sl])
            j = int(jpos[sl])
            parts = []
            while rem > 0:
                t = pos // 128
                p0 = pos % 128
                assert p0 in (0, 64), f"illegal partition base {p0}"
                k = min(128 - p0, rem)
                if p0 == 64:
                    k = min(k, 64)
                pos += k
                rem -= k
                parts.append((t, p0, k))
            # rotate a p0==0 part to the front (group must start at
            # partition base 0)
            i0 = next((i for i, e in enumerate(parts) if e[1] == 0), None)
            if i0 is None:
                nzero += 1
                need_zero = True
            else:
                parts = parts[i0:i0 + 1] + parts[:i0] + parts[i0 + 1:]
                need_zero = False
            n = len(parts)
            for i, (t, p0, k) in enumerate(parts):
                first = (i == 0) and not need_zero
                slist.append((t, p0, k, j, first, i == n - 1,
                              need_zero and i == 0))
        segs.append(slist)

    chunks = []
    s0 = 0
    while s0 < NSTRIP:
        s1 = s0
        cols = 0
        while s1 < NSTRIP and cols + strip_cols[s1] <= GCMAX:
            cols += strip_cols[s1]
            s1 += 1
        assert s1 > s0, "single strip exceeds GCMAX"
        chunks.append((s0, s1, int(col_base[s0]), int(cols)))
        s0 = s1
    return dict(slot_off=slot_off, strip_cols=strip_cols, col_base=col_base,
                jpos=jpos, F=F, segs=segs, chunks=chunks, nzero=nzero)


def _build_program(lay):
    import concourse.bacc as bacc
    import concourse.mybir as mybir
    from concourse.tile import TileContext

    dt = mybir.dt
    Act = mybir.ActivationFunctionType
    Alu = mybir.AluOpType

    F = lay["F"]
    nc = bacc.Bacc("TRN2", target_bir_lowering=False, debug=False)

    # dcq2: per chunk, [dcy cols | dcx cols] pair-duplicated -> [P, 2F, 2]
    dcq2 = nc.dram_tensor("dcq2", [P, 2 * F, 2], dt.int16,
                          kind="ExternalInput")
    # vnb2: F columns pair-duplicated -> [P, F, 2]
    vnb2 = nc.dram_tensor("vnb2", [P, F, 2], dt.bfloat16,
                          kind="ExternalInput")
    iot = nc.dram_tensor("iot", [P, 18, 2], dt.int16, kind="ExternalInput")
    out = nc.dram_tensor("out", [NSTRIP, PW, 8 * PW], dt.float32,
                         kind="ExternalOutput")

    SC = float(math.sqrt(2.0) / 256.0)

    with TileContext(nc) as tc:
        with (
            tc.tile_pool(name="io", bufs=1) as io,
            tc.tile_pool(name="prof", bufs=1) as prof,
            tc.tile_pool(name="stage", bufs=8) as stage,
            tc.tile_pool(name="psum", bufs=8, space="PSUM") as psum,
        ):
            t_dcq2 = io.tile([P, 2 * F, 2], dt.int16)
            t_vnb2 = io.tile([P, F, 2], dt.bfloat16)
            t_iot = io.tile([P, 18, 2], dt.int16)
            t_zro = io.tile([P, PW], dt.bfloat16)
            nc.sync.dma_start(out=t_dcq2[:], in_=dcq2[:])
            nc.sync.dma_start(out=t_vnb2[:], in_=vnb2[:])
            nc.sync.dma_start(out=t_iot[:], in_=iot[:])
            nc.gpsimd.memset(t_zro[:], 0.0)

            def emit_mm(ck):
                """Matmuls for a chunk's strips; k=64 segments clustered
                last per strip (PE tile-config switch amortization)."""
                (s0, s1, c0, gc), pr, colp = ck
                strips = []
                for s in range(s0, s1):
                    strip = psum.tile([PW, 8 * PW], dt.float32,
                                      tag="strip", name="strip")
                    base = int(lay["col_base"][s]) - c0
                    for (t, p0, k, j, first, last, zopen) in lay["segs"][s]:
                        tl = base + t
                        if zopen:
                            nc.tensor.matmul(
                                out=strip[:, j * PW:(j + 1) * PW],
                                lhsT=t_zro[0:1, :],
                                rhs=t_zro[0:1, :],
                                start=True, stop=False)
                        nc.tensor.matmul(
                            out=strip[:, j * PW:(j + 1) * PW],
                            lhsT=pr[p0:p0 + k, tl],
                            rhs=colp[p0:p0 + k, tl],
                            start=first, stop=last)
                    strips.append((s, strip))
                return strips

            ncopy = [0]

            def emit_copy(strips):
                """Strip copies (alternating DVE/ACT) + output DMAs."""
                for s, strip in strips:
                    st = stage.tile([PW, 8 * PW], dt.float32,
                                    tag="st", name="st")
                    if ncopy[0] % 8 != 7:
                        nc.vector.tensor_copy(out=st[:], in_=strip[:])
                    else:
                        nc.scalar.copy(out=st[:], in_=strip[:])
                    ncopy[0] += 1
                    nc.sync.dma_start(out=out[s], in_=st[:])

            co2 = 0                     # running column offset into dcq2
            prev = None                 # software pipeline: mm lags 1 chunk
            pcopy = None                # copies lag 2 chunks
            for (s0, s1, c0, gc) in lay["chunks"]:
                d = prof.tile([P, 2 * gc, 18, 2], dt.int16, tag="d", bufs=2,
                              name=f"d{gc}")
                nc.vector.tensor_tensor(
                    out=d[:],
                    in0=t_iot[:, None, :, :].to_broadcast([P, 2 * gc, 18, 2]),
                    in1=t_dcq2[:, co2:co2 + 2 * gc, None, :]
                        .to_broadcast([P, 2 * gc, 18, 2]),
                    op=Alu.subtract)
                pr = prof.tile([P, 2 * gc, 18, 2], dt.bfloat16, tag="pr",
                               bufs=3, name=f"pr{gc}")
                nc.scalar.activation(out=pr[:], in_=d[:],
                                     func=Act.Derivative_Erf, scale=SC)
                colp = prof.tile([P, gc, 18, 2], dt.bfloat16, tag="colp",
                                 bufs=3, name=f"colp{gc}")
                # multiply split DVE/Pool for engine balance (by column)
                g1 = max(0, min(gc, int(round(gc * MUL_DVE))))
                if g1 > 0:
                    nc.vector.tensor_tensor(
                        out=colp[:, :g1], in0=pr[:, gc:gc + g1],
                        in1=t_vnb2[:, c0:c0 + g1, None, :]
                            .to_broadcast([P, g1, 18, 2]),
                        op=Alu.mult)
                if g1 < gc:
                    nc.gpsimd.tensor_tensor(
                        out=colp[:, g1:], in0=pr[:, gc + g1:2 * gc],
                        in1=t_vnb2[:, c0 + g1:c0 + gc, None, :]
                            .to_broadcast([P, gc - g1, 18, 2]),
                        op=Alu.mult)

                if pcopy is not None:
                    emit_copy(pcopy)
                    pcopy = None
                if prev is not None:
                    pcopy = emit_mm(prev)
                prev = ((s0, s1, c0, gc), pr, colp)
                co2 += 2 * gc
            if pcopy is not None:
                emit_copy(pcopy)
            emit_copy(emit_mm(prev))
    nc.compile()
    from concourse.bass_interp import get_hw_module
    nc.m = get_hw_module(nc.m)
    return nc


def _host_shard(x, y, values, plan, lay):
    from ml_dtypes import bfloat16

    xp = (x.astype(np.float64) + 1.0) * (WIDTH / 2.0)
    yp = (y.astype(np.float64) + 1.0) * (HEIGHT / 2.0)
    xb = np.clip(np.floor(xp).astype(np.int64), 0, WIDTH - 1)
    yb = np.clip(np.floor(yp).astype(np.int64), 0, HEIGHT - 1)
    bcx = xb // BLK
    bry = yb // BLK
    gb = bry * GRID + bcx
    core = plan["core_of"][gb]
    slot = plan["slot_of"][gb]
    dcxq_all = np.round((xp - (bcx * BLK - 2)) * 256).astype(np.int16)
    dcyq_all = np.round((yp - (bry * BLK - 2)) * 256).astype(np.int16)
    fxq = (dcxq_all.astype(np.int64) % 256) / 256.0
    fyq = (dcyq_all.astype(np.int64) % 256) / 256.0
    vnorm = (values.astype(np.float64)
             / (2.0 * (1.0 + _Q2 * np.cos(2 * np.pi * fxq))
                * (1.0 + _Q2 * np.cos(2 * np.pi * fyq))))

    F = lay["F"]
    # global slot slab base: strip col_base*128 + slot_off
    slab = lay["col_base"][slot // 8] * 128 + lay["slot_off"][slot]

    iota_a = np.broadcast_to((np.arange(PW, dtype=np.int16) * 256)[None, :],
                             (P, PW)).reshape(P, 18, 2).copy()

    in_maps = []
    for c in range(N_CORES):
        m = core == c
        ps = slot[m]
        order = np.argsort(ps, kind="stable")
        ps = ps[order]
        counts = np.bincount(ps, minlength=NSLOT)
        if (counts > plan["caps"]).any():
            raise RuntimeError("slot overflow vs caps")
        starts = np.zeros(NSLOT, np.int64)
        np.cumsum(counts[:-1], out=starts[1:])
        idx = np.arange(ps.size) - starts[ps]
        dst = slab[m][order] + idx

        ya = np.full(F * P, 18 * 256, np.int16)
        xa = np.full(F * P, 18 * 256, np.int16)
        va = np.zeros(F * P, np.float64)
        ya[dst] = dcyq_all[m][order]
        xa[dst] = dcxq_all[m][order]
        va[dst] = vnorm[m][order]

        yaT = ya.reshape(F, P).T            # [P, F]
        xaT = xa.reshape(F, P).T
        vaT = va.reshape(F, P).T

        # dcq2: per chunk [dcy cols | dcx cols], each value duplicated
        dcq2_a = np.empty((P, 2 * F, 2), np.int16)
        off = 0
        for (_, _, c0, gc) in lay["chunks"]:
            dcq2_a[:, off:off + gc, 0] = yaT[:, c0:c0 + gc]
            dcq2_a[:, off + gc:off + 2 * gc, 0] = xaT[:, c0:c0 + gc]
            off += 2 * gc
        dcq2_a[:, :, 1] = dcq2_a[:, :, 0]

        vnb2_a = np.empty((P, F, 2), bfloat16)
        vnb2_a[:, :, 0] = vaT.astype(bfloat16)
        vnb2_a[:, :, 1] = vnb2_a[:, :, 0]
        in_maps.append({"dcq2": dcq2_a, "vnb2": np.ascontiguousarray(vnb2_a),
                        "iot": iota_a})
    return in_maps


def _assemble(results, plan, lay):
    img = np.zeros((HEIGHT + 4, WIDTH + 4), np.float64)
    jpos = lay["jpos"]
    for c in range(N_CORES):
        strips = results[c]["out"]          # [NSTRIP, PW, 8*PW]
        for sl in range(NSLOT):
            gb = plan["inv"][c, sl]
            bry, bcx = divmod(int(gb), GRID)
            j = int(jpos[sl])
            patch = strips[sl // 8, :, j * PW:(j + 1) * PW]
            img[bry * BLK:bry * BLK + PW, bcx * BLK:bcx * BLK + PW] += patch
    return img[2:2 + HEIGHT, 2:2 + WIDTH].astype(np.float32)


def kernel(x, y, values):
    global _COMPILED
    if _COMPILED is None:
        plan = _block_plan(x, y)
        lay = _layout_from_caps(plan["caps"])
        nc = _build_program(lay)
        _COMPILED = (nc, plan, lay)
    nc, plan, lay = _COMPILED
    in_maps = _host_shard(x, y, values, plan, lay)
    from concourse.bass_utils import run_bass_kernel_spmd
    import os
    trace = bool(int(os.environ.get("SPLAT_TRACE", "0")))
    res = run_bass_kernel_spmd(nc, in_maps, list(range(N_CORES)), trace=trace)
    kernel.last_exec_time_ns = res.exec_time_ns
    kernel.last_results = res
    return _assemble(res.results, plan, lay)


kernel.last_exec_time_ns = None
